# revision 3
# baseline (speedup 1.0000x reference)
import numpy as np

B = 2
T = 2048
D = 1024
H = 16
HD = 64
HPC = 4
M = HPC * HD
NCORES = 8
TQB = 512
NKC = D // 128
NTB = T // TQB
NTC = T // 128

_CACHE = {}


def _build():
    import concourse.bass as bass
    import concourse.mybir as mybir
    import concourse.tile as tile
    from concourse import bacc

    F32 = mybir.dt.float32
    F16 = mybir.dt.float16

    nc = bacc.Bacc("TRN2", target_bir_lowering=False, debug=False, num_devices=NCORES)
    xT = nc.dram_tensor("xT", [D, T], F16, kind="ExternalInput")
    wqT = nc.dram_tensor("wqT", [D, M], F16, kind="ExternalInput")
    wkT = nc.dram_tensor("wkT", [D, M], F16, kind="ExternalInput")
    wvT = nc.dram_tensor("wvT", [D, M], F16, kind="ExternalInput")
    woT = nc.dram_tensor("woT", [M, D], F16, kind="ExternalInput")
    msk = nc.dram_tensor("msk", [4, 128, TQB], F32, kind="ExternalInput")
    outT = nc.dram_tensor("outT", [D, T], F32, kind="ExternalOutput")

    with tile.TileContext(nc) as tc:
        persist = tc.alloc_tile_pool(name="persist", bufs=1)
        xt_sb = persist.tile([128, NKC, T], F16, name="xt_sb")
        wq_sb = persist.tile([128, NKC, M], F16, name="wq_sb")
        wk_sb = persist.tile([128, NKC, M], F16, name="wk_sb")
        wv_sb = persist.tile([128, NKC, M], F16, name="wv_sb")
        wo_sb = persist.tile([128, 2, D], F16, name="wo_sb")
        mask_sb = persist.tile([128, 4, TQB], F32, name="mask_sb")
        qT_sb = persist.tile([128, 2, T], F16, name="qT_sb")
        kT_sb = persist.tile([128, 2, T], F16, name="kT_sb")
        v_sb = persist.tile([128, NTC, HPC * (HD + 1)], F16, name="v_sb")
        oT_sb = persist.tile([128, 2, T], F16, name="oT_sb")

        for c in range(NKC):
            nc.sync.dma_start(out=xt_sb[:, c, :], in_=xT[c * 128:(c + 1) * 128, :])
            nc.sync.dma_start(out=wq_sb[:, c, :], in_=wqT[c * 128:(c + 1) * 128, :])
            nc.sync.dma_start(out=wk_sb[:, c, :], in_=wkT[c * 128:(c + 1) * 128, :])
            nc.sync.dma_start(out=wv_sb[:, c, :], in_=wvT[c * 128:(c + 1) * 128, :])
        for j in range(2):
            nc.sync.dma_start(out=wo_sb[:, j, :], in_=woT[j * 128:(j + 1) * 128, :])
        for r in range(4):
            nc.sync.dma_start(out=mask_sb[:, r, :], in_=msk[r, :, :])

        v_heads = v_sb.rearrange("p c (h e) -> p c h e", e=HD + 1)
        nc.vector.memset(v_heads[:, :, :, HD:HD + 1], 1.0)

        ps1 = tc.alloc_tile_pool(name="ps1", bufs=4, space="PSUM")
        for w_sb, dst in ((wq_sb, qT_sb), (wk_sb, kT_sb)):
            for mc in range(2):
                psums = []
                for tb in range(NTB):
                    pj = ps1.tile([128, TQB], F32, name=f"pj{mc}{tb}", tag="pj")
                    psums.append(pj)
                for kc in range(NKC):
                    for tb in range(NTB):
                        nc.tensor.matmul(
                            psums[tb][:, :],
                            w_sb[:, kc, mc * 128:(mc + 1) * 128],
                            xt_sb[:, kc, tb * TQB:(tb + 1) * TQB],
                            start=(kc == 0),
                            stop=(kc == NKC - 1),
                        )
                for tb in range(NTB):
                    nc.vector.tensor_copy(
                        dst[:, mc, tb * TQB:(tb + 1) * TQB], psums[tb][:, :]
                    )
        for t_ in range(NTC):
            pv = ps1.tile([128, M], F32, name="pv", tag="pj")
            for kc in range(NKC):
                nc.tensor.matmul(
                    pv[:, :],
                    xt_sb[:, kc, t_ * 128:(t_ + 1) * 128],
                    wv_sb[:, kc, :],
                    start=(kc == 0),
                    stop=(kc == NKC - 1),
                )
            nc.vector.tensor_copy(
                v_heads[:, t_, :, 0:HD],
                pv[:, :].rearrange("p (h d) -> p h d", d=HD),
            )
        ps1.release()

        ps_s = tc.alloc_tile_pool(name="ps_s", bufs=2, space="PSUM")
        ps_o = tc.alloc_tile_pool(name="ps_o", bufs=2, space="PSUM")
        ps_u = tc.alloc_tile_pool(name="ps_u", bufs=2, space="PSUM")
        pp = tc.alloc_tile_pool(name="pp", bufs=3)
        sm = tc.alloc_tile_pool(name="sm", bufs=2)

        for b in range(NTB):
            tq = slice(b * TQB, (b + 1) * TQB)
            nchunks = 4 * b + 4
            for hp in range(2):
                o_ps = [
                    ps_o.tile([HD + 1, TQB], F32, name=f"o{j}", tag="o")
                    for j in range(2)
                ]
                for c in range(nchunks):
                    s_ps = ps_s.tile([128, 2 * TQB], F32, name="s_ps", tag="s")
                    for j in range(2):
                        h = hp * 2 + j
                        base = (h % 2) * 64
                        nc.tensor.matmul(
                            s_ps[:, j * TQB:(j + 1) * TQB],
                            kT_sb[base:base + 64, h // 2, c * 128:(c + 1) * 128],
                            qT_sb[base:base + 64, h // 2, tq],
                            start=True,
                            stop=True,
                        )
                    if c >= 4 * b:
                        r = c - 4 * b
                        mrow = mask_sb[:, r, :]
                        mb = bass.AP(
                            tensor=mrow.tensor,
                            offset=mrow.offset,
                            ap=[mrow.ap[0], [0, 2], mrow.ap[-1]],
                        )
                        sv = s_ps[:, :].rearrange("p (j q) -> p j q", q=TQB)
                        nc.vector.tensor_add(sv, sv, mb)
                    p_t = pp.tile([128, 2 * TQB], F16, name="p_t", tag="p")
                    nc.scalar.activation(
                        out=p_t[:, :],
                        in_=s_ps[:, :],
                        func=mybir.ActivationFunctionType.Exp,
                        scale=0.125,
                    )
                    for j in range(2):
                        h = hp * 2 + j
                        nc.tensor.matmul(
                            o_ps[j][:, :],
                            v_sb[:, c, h * (HD + 1):(h + 1) * (HD + 1)],
                            p_t[:, j * TQB:(j + 1) * TQB],
                            start=(c == 0),
                            stop=(c == nchunks - 1),
                        )
                for j in range(2):
                    h = hp * 2 + j
                    rrow = sm.tile([HD + 1, TQB], F32, name="rrow", tag="rrow")
                    nc.vector.tensor_copy(rrow[HD:HD + 1, :], o_ps[j][HD:HD + 1, :])
                    r0 = sm.tile([1, TQB], F32, name="r0", tag="r0")
                    nc.sync.dma_start(out=r0[0:1, :], in_=rrow[HD:HD + 1, :])
                    inv0 = sm.tile([1, TQB], F32, name="inv0", tag="inv0")
                    nc.vector.reciprocal(inv0[0:1, :], r0[0:1, :])
                    inv_b = sm.tile([HD, TQB], F32, name="inv_b", tag="invb")
                    nc.gpsimd.partition_broadcast(inv_b[:, :], inv0[0:1, :])
                    if h % 2 == 0:
                        nc.vector.tensor_mul(
                            oT_sb[0:HD, h // 2, tq], o_ps[j][0:HD, :], inv_b[:, :]
                        )
                    else:
                        otmp = sm.tile([HD, TQB], F16, name="otmp", tag="otmp")
                        nc.vector.tensor_mul(otmp[:, :], o_ps[j][0:HD, :], inv_b[:, :])
                        nc.sync.dma_start(
                            out=oT_sb[64:128, h // 2, tq], in_=otmp[:, :]
                        )
            for ic in range(NKC):
                u_ps = ps_u.tile([128, TQB], F32, name="u_ps", tag="u")
                for mc in range(2):
                    nc.tensor.matmul(
                        u_ps[:, :],
                        wo_sb[:, mc, ic * 128:(ic + 1) * 128],
                        oT_sb[:, mc, tq],
                        start=(mc == 0),
                        stop=(mc == 1),
                    )
                u_sb = sm.tile([128, TQB], F32, name="u_sb", tag="usb")
                nc.vector.tensor_copy(u_sb[:, :], u_ps[:, :])
                nc.sync.dma_start(
                    out=outT[ic * 128:(ic + 1) * 128, tq], in_=u_sb[:, :]
                )
        sm.release()
        pp.release()
        ps_u.release()
        ps_o.release()
        ps_s.release()
        persist.release()
    nc.compile()
    return nc


def get_nc():
    if "nc" not in _CACHE:
        _CACHE["nc"] = _build()
    return _CACHE["nc"]


def make_in_maps(x, Wq, Wk, Wv, Wo):
    x = np.asarray(x, dtype=np.float32)
    Wq = np.asarray(Wq, dtype=np.float32)
    Wk = np.asarray(Wk, dtype=np.float32)
    Wv = np.asarray(Wv, dtype=np.float32)
    Wo = np.asarray(Wo, dtype=np.float32)

    masks = np.zeros((4, 128, TQB), dtype=np.float32)
    tk = np.arange(128)[:, None]
    tqi = np.arange(TQB)[None, :]
    for r in range(4):
        masks[r] = np.where(tk <= tqi - 128 * r, 0.0, -1e9).astype(np.float32)

    in_maps = []
    for c in range(NCORES):
        b = c // 4
        hg = c % 4
        ms = slice(hg * M, (hg + 1) * M)
        in_maps.append(
            {
                "xT": np.ascontiguousarray(x[b].T).astype(np.float16),
                "wqT": np.ascontiguousarray(Wq[ms, :].T).astype(np.float16),
                "wkT": np.ascontiguousarray(Wk[ms, :].T).astype(np.float16),
                "wvT": np.ascontiguousarray(Wv[ms, :].T).astype(np.float16),
                "woT": np.ascontiguousarray(Wo[:, ms].T).astype(np.float16),
                "msk": masks,
            }
        )
    return in_maps


def combine_outputs(results):
    out = np.zeros((B, T, D), dtype=np.float32)
    for c in range(NCORES):
        out[c // 4] += results[c]["outT"].T
    return out


def kernel(x, Wq, Wk, Wv, Wo):
    from concourse.bass_utils import run_bass_kernel_spmd

    nc = get_nc()
    in_maps = make_in_maps(x, Wq, Wk, Wv, Wo)
    res = run_bass_kernel_spmd(nc, in_maps, core_ids=list(range(NCORES)))
    return combine_outputs(res.results)


if __name__ == "__main__":
    rng = np.random.default_rng(0)
    s = 1.0 / np.sqrt(D)
    x = rng.standard_normal((B, T, D), dtype=np.float32)
    Wq = rng.standard_normal((D, D), dtype=np.float32) * s
    Wk = rng.standard_normal((D, D), dtype=np.float32) * s
    Wv = rng.standard_normal((D, D), dtype=np.float32) * s
    Wo = rng.standard_normal((D, D), dtype=np.float32) * s
    out = kernel(x, Wq, Wk, Wv, Wo)
    print("kernel ran, out shape", out.shape, "mean", float(np.abs(out).mean()))
